# revision 57
# baseline (speedup 1.0000x reference)
"""TRN2 Bass kernel for nn_MultiHeadSeqAttention (B=8, M=1024, H=1024, 16 heads).

Reference computes out = ((h Wq^T) (h Wk^T)^T) (h Wv^T) per head, then Wo^T.
No softmax, so the product reassociates per head h:
    out_h = q_h @ (k_h^T v_h)      with  k_h^T v_h = Wk_h (h^T h) Wv_h^T
Route through the Gram matrix G = h^T h, which is SYMMETRIC:
    G  = hN^T hN          upper-triangle windows only: 72 MM-equivalents
                          (28 lower [128,128] blocks via 7 batched DMA-XBAR
                          transposes on the otherwise idle scalar queue)
    AT = G @ wk           [j, o], 128 MMs                  (wk = Wk^T)
    S'_h = wv_h^T AT_h    per-head [64,64], head-pair packed, 64 small MMs
    qT = wq^T ht          128 MMs
    R_h = S_h @ wo_h      32 quadrant-paired MMs   (folds Wo into S)
    out = qT^T @ R        128 MMs
vs the direct route (k, v, q, Wo GEMMs = 512 MMs + smalls) this saves ~56
big-MM equivalents (~12us PE) per core.

Schedule notes (measured, from trace analysis):
 - every matmul lives in a same-PSUM-bank accumulation group so LDWEIGHTS
   hides behind the previous matmul (216ns/MM); per-MM bank cycling
   measured 474ns/MM (LDW exposed + HAM oscillation).
 - all input tensors stream on the sync queue as 7 descriptors in
   first-use order (hn halves, wk, wv, wq, ht, wo). A solo ring sustains
   ~235-280 GB/s; concurrent rings split unevenly (~240/110) and starve
   whoever is second; DMA elements >16KB run at ~105 GB/s (keep <=16KB).
   The completion-semaphore pool is ~10 deep and recycles in global
   emission order, so descriptor count stays minimal and the transposes
   recycle only early-completing hn semaphores.
 - AT visits jb=7 first: it needs no transposed G blocks and its S'
   matmuls (first touch of wv) land just after the wv load completes.
 - G pass A is paced by the two hn half-arrivals (4-MM bursts per bank);
   a const-operand warmup burst covers the HAM clock ramp until then.
 - R runs as one dense block after qT (interleaving cost ~1us); its
   PSUM evacuation alternates scalar/vector so casts never pace it.
 - output is bf16 (host casts back to f32), halving the store traffic;
   the last tile leaves as four [128,256] chunks in separate PSUM banks
   so the final cast+store chain is short.

Sharding: data-parallel over B across 8 cores; no collectives.
Precision: bf16 operands, fp32 PSUM accumulation; absmax rel err ~5.0e-3.
"""

import numpy as np
import ml_dtypes

import concourse.bass as bass
import concourse.mybir as mybir
import concourse.tile as tile
from concourse import bacc
from concourse.bass_utils import run_bass_kernel_spmd

F32 = mybir.dt.float32
BF16 = mybir.dt.bfloat16
COPY = mybir.ActivationFunctionType.Copy

P = 128          # partitions
H = 1024         # model dim
M = 1024         # sequence length
NT = H // P      # 8 tiles of 128
D = 64           # head dim
NC = 8           # cores
FD = 512         # matmul moving free dim (one PSUM bank of fp32)
WARMUP_MM = 10   # const-operand matmuls; cold pace fills until hn lands

_CACHE = {}


def _build():
    nc = bacc.Bacc("TRN2", target_bir_lowering=False, debug=False,
                   num_devices=NC, enable_asserts=False)

    hn_d = nc.dram_tensor("hn", [M, H], BF16, kind="ExternalInput")
    wkb_d = nc.dram_tensor("wkb", [P, NT * H], BF16, kind="ExternalInput")
    wvb_d = nc.dram_tensor("wvb", [P, NT * H], BF16, kind="ExternalInput")
    wqb_d = nc.dram_tensor("wqb", [P, NT * H], BF16, kind="ExternalInput")
    htb_d = nc.dram_tensor("htb", [P, NT * H], BF16, kind="ExternalInput")
    wob_d = nc.dram_tensor("wob", [P, NT * H], BF16, kind="ExternalInput")
    out_d = nc.dram_tensor("out", [M, H], BF16, kind="ExternalOutput")

    with tile.TileContext(nc) as tc:
        with tc.tile_pool(name="sb", bufs=1) as sb, \
             tc.tile_pool(name="ps", bufs=1, space="PSUM") as ps:

            # ---- warmup: dep-free matmuls; ends about when hn half 0 lands
            wu_lhs = nc.const_aps.tensor(1.0, [P, P], BF16)
            wu_rhs = nc.const_aps.tensor(1.0, [P, FD], BF16)
            wu_ps = ps.tile([P, FD], F32, tag="big", bufs=8, name="wu_ps")
            for _ in range(WARMUP_MM):
                nc.tensor.matmul(wu_ps[:], wu_lhs, wu_rhs,
                                 start=True, stop=True)
            # finer-grained tail so the handoff to the first G matmul
            # overshoots hn arrival by at most ~110ns
            for _ in range(6):
                nc.tensor.matmul(wu_ps[:, 0:P], wu_lhs, wu_rhs[:, 0:P],
                                 start=True, stop=True)

            # ---- loads: all sequential on the sync ring in first-use
            # order; the scalar ring is reserved for the G transposes ----
            hnBig = sb.tile([P, NT * H], BF16, tag="hn", name="hn")
            for r0, r1 in ((0, 512), (512, 1024)):
                src = hn_d.ap()[r0:r1, :].rearrange("(t p) c -> p t c", p=P)
                dst = hnBig[:, H * r0 // P:H * r1 // P] \
                    .rearrange("p (t c) -> p t c", c=H)
                nc.sync.dma_start(dst, src)

            def hn(s):
                return hnBig[:, H * s:H * s + H]

            def big_load(dram, tag, eng):
                t = sb.tile([P, NT * H], BF16, tag=tag, name=tag)
                eng.dma_start(t[:], dram.ap()[:, :])
                return t

            wkB = big_load(wkb_d, "wkB", nc.sync)
            # wv as two separate TILES (separate completion semaphores),
            # upper half first: the AT phase visits jb=7 first and its S'
            # matmuls read wv cols 7168+ right as that half lands
            wvHi = sb.tile([P, 4 * H], BF16, tag="wvHi", name="wvHi")
            nc.sync.dma_start(wvHi[:], wvb_d.ap()[:, 4 * H:NT * H])
            wvLo = sb.tile([P, 4 * H], BF16, tag="wvLo", name="wvLo")
            nc.sync.dma_start(wvLo[:], wvb_d.ap()[:, 0:4 * H])
            # wq/ht/wo are emitted AFTER the G transposes (below) so the
            # transposes' completion semaphores recycle only from the five
            # early loads; these three have >=15us of deadline slack

            # ---- phase G: upper-triangle windows of G = hN^T hN.
            # Row-chunk ib keeps columns [128*ib, 1024) as (up to) two PSUM
            # windows. Pass A = ib 0-3 (8 banks), 4-MM bursts per bank paced
            # by the hn halves; pass B = ib 4-7 (4 banks), dense. ----
            gt = sb.tile([P, NT * H], BF16, tag="gt", name="gt")
            # midpoint split keeps every window >=320 cols so LDWEIGHTS
            # (107ns) always hides under the matmul stream (>=133ns);
            # a 512/(512-128*ib) split leaves the narrow window LDW-bound
            WIN_A = []
            for ib in range(4):
                half = (H - P * ib) // 2 // 64 * 64
                WIN_A.append((ib, P * ib, half))
                WIN_A.append((ib, P * ib + half, H - P * ib - half))
            gpsA = {}
            for (ib, c0, w) in WIN_A:
                gpsA[(ib, c0)] = ps.tile([P, w], F32, tag="big", bufs=8,
                                         name=f"gA{ib}_{c0}")
            for s_range in (range(0, 4), range(4, 8)):
                for (ib, c0, w) in WIN_A:
                    for s in s_range:
                        nc.tensor.matmul(
                            gpsA[(ib, c0)][:],
                            hn(s)[:, P * ib:P * ib + P],
                            hn(s)[:, c0:c0 + w],
                            start=(s == 0), stop=(s == NT - 1),
                            skip_group_check=True,
                        )
            for (ib, c0, w) in WIN_A:
                nc.vector.tensor_copy(gt[:, H * ib + c0:H * ib + c0 + w],
                                      gpsA[(ib, c0)][:])
            for ib in range(4, 8):
                c0, w = P * ib, H - P * ib
                pt = ps.tile([P, w], F32, tag="big", bufs=8, name=f"gB{ib}")
                for s in range(NT):
                    nc.tensor.matmul(
                        pt[:],
                        hn(s)[:, P * ib:P * ib + P],
                        hn(s)[:, c0:c0 + w],
                        start=(s == 0), stop=(s == NT - 1),
                    )
                nc.vector.tensor_copy(gt[:, H * ib + c0:H * ib + c0 + w],
                                      pt[:])

            # lower-triangle blocks (b, a), b > a: batched XBAR transposes on
            # the scalar queue, one instruction per source row-chunk a,
            # ordered by when the AT phase consumes them (jb visit order).
            gt3 = gt[:].rearrange("p (b c) -> p b c", c=H)
            for a in (0, 1, 2, 3, 4, 5, 6):
                nc.scalar.dma_start(
                    gt3[:, a + 1:NT, P * a:P * a + P],
                    gt[:, H * a + P * (a + 1):H * a + H],
                    transpose=True)

            wqB = big_load(wqb_d, "wqB", nc.sync)
            htB = big_load(htb_d, "htB", nc.sync)
            woB = big_load(wob_d, "woB", nc.sync)

            # ---- phase AT + S': AT = G @ wk; S' pairs accumulate after each
            # at-tile half is cast. jb=7 needs no transposed blocks; jb 3..0
            # need pass-A-sourced transposes, jb 6..4 pass-B-sourced. ----
            s_psA = ps.tile([P, FD], F32, tag="big", bufs=8, name="s_psA")
            s_psB = ps.tile([P, FD], F32, tag="big", bufs=8, name="s_psB")
            nc.vector.memset(s_psA[:], 0.0)
            nc.vector.memset(s_psB[:], 0.0)

            # jb=7 first: it needs no transposed blocks, and its S' matmuls
            # (the first to touch wv) land just after the wv load completes
            AT_ORDER = [7, 3, 2, 1, 0, 6, 5, 4]
            for idx, jb in enumerate(AT_ORDER):
                a_t = sb.tile([P, H], BF16, tag="at", bufs=3, name=f"at{jb}")
                for oc in range(2):
                    p_t = ps.tile([P, FD], F32, tag="big", bufs=8,
                                  name=f"pa{jb}{oc}")
                    for ib in range(NT):
                        nc.tensor.matmul(
                            p_t[:],
                            gt[:, H * ib + P * jb:H * ib + P * jb + P],
                            wkB[:, H * ib + FD * oc:H * ib + FD * oc + FD],
                            start=(ib == 0), stop=(ib == NT - 1),
                        )
                    nc.vector.tensor_copy(a_t[:, FD * oc:FD * oc + FD], p_t[:])
                    for g in range(4 * oc, 4 * oc + 4):
                        bank = s_psA if g < 4 else s_psB
                        cc = P * (g % 4)
                        nc.tensor.matmul(
                            bank[:, cc:cc + P],
                            (wvHi if jb >= 4 else wvLo)
                            [:, H * (jb % 4) + P * g:H * (jb % 4) + P * g + P],
                            a_t[:, P * g:P * g + P],
                            start=False, stop=(idx == NT - 1),
                            skip_group_check=True,
                        )
            s_sbA = sb.tile([P, FD], BF16, tag="ssb", bufs=2, name="s_sbA")
            s_sbB = sb.tile([P, FD], BF16, tag="ssb", bufs=2, name="s_sbB")
            nc.scalar.activation(s_sbA[:], s_psA[:], COPY)
            nc.scalar.activation(s_sbB[:], s_psB[:], COPY)

            # ---- phase qT + R: qT = wq^T ht; R_g folds Wo into S per head
            # pair (quadrant-packed 64-part matmuls), spread between q tiles ----
            qt = [None] * NT
            rstack = [None] * NT

            def emit_q(to):
                q_t = sb.tile([P, M], BF16, tag=f"qt{to}", name=f"qt{to}")
                for cm in range(2):
                    p_t = ps.tile([P, FD], F32, tag="big", bufs=8,
                                  name=f"pq{to}{cm}")
                    for ci in range(NT):
                        nc.tensor.matmul(
                            p_t[:],
                            wqB[:, H * ci + P * to:H * ci + P * to + P],
                            htB[:, H * ci + FD * cm:H * ci + FD * cm + FD],
                            start=(ci == 0), stop=(ci == NT - 1),
                        )
                    nc.vector.tensor_copy(q_t[:, FD * cm:FD * cm + FD], p_t[:])
                qt[to] = q_t

            def emit_r(g):
                r_t = sb.tile([P, H], BF16, tag=f"rs{g}", name=f"rs{g}")
                sbank = s_sbA if g < 4 else s_sbB
                cc = P * (g % 4)
                for jc in range(2):
                    p_t = ps.tile([P, FD], F32, tag="big", bufs=8,
                                  name=f"pr{g}{jc}")
                    for hh in range(2):
                        pb = D * hh
                        nc.tensor.matmul(
                            p_t[pb:pb + D, :],
                            sbank[pb:pb + D, cc + pb:cc + pb + D],
                            woB[pb:pb + D, H * g + FD * jc:H * g + FD * jc + FD],
                            start=True, stop=True,
                        )
                    if jc:
                        nc.vector.tensor_copy(
                            r_t[:, FD * jc:FD * jc + FD], p_t[:])
                    else:
                        nc.scalar.activation(r_t[:, FD * jc:FD * jc + FD],
                                             p_t[:], COPY)
                rstack[g] = r_t

            # R as one dense block after qT: interleaving it between q tiles
            # measured ~1us of group-switch overhead
            for to in range(NT):
                emit_q(to)
            for g in range(NT):
                emit_r(g)

            # ---- phase out: out = qT^T @ R. Stores are paired (two m-tiles
            # per descriptor) to keep the DMA instruction count low; the
            # last tile goes out as quarter chunks so the end of kernel
            # gates on a 64KB transfer. ----
            for tp in range(NT // 2):
                o_sb = sb.tile([P, 2 * H], BF16, tag="ot", bufs=2,
                               name=f"o{tp}")
                for ti in range(2):
                    tm = 2 * tp + ti
                    if tm == NT - 1:
                        # last tile: cj=0 as a normal [128,512] group+store;
                        # cj=1 as two [128,256] groups in separate PSUM
                        # banks so the final cast+store chain is short
                        p_t = ps.tile([P, FD], F32, tag="big", bufs=8,
                                      name="pf7c0")
                        for to in range(NT):
                            nc.tensor.matmul(
                                p_t[:],
                                qt[to][:, P * tm:P * tm + P],
                                rstack[to][:, 0:FD],
                                start=(to == 0), stop=(to == NT - 1),
                            )
                        nc.vector.tensor_copy(
                            o_sb[:, H * ti:H * ti + FD], p_t[:])
                        nc.sync.dma_start(
                            out_d.ap()[P * tm:P * tm + P, 0:FD],
                            o_sb[:, H * ti:H * ti + FD])
                        for qq in range(2):
                            c0 = FD + 256 * qq
                            p_t = ps.tile([P, 256], F32, tag="big", bufs=8,
                                          name=f"pf7q{qq}")
                            for to in range(NT):
                                nc.tensor.matmul(
                                    p_t[:],
                                    qt[to][:, P * tm:P * tm + P],
                                    rstack[to][:, c0:c0 + 256],
                                    start=(to == 0), stop=(to == NT - 1),
                                )
                            nc.vector.tensor_copy(
                                o_sb[:, H * ti + c0:H * ti + c0 + 256],
                                p_t[:])
                            (nc.scalar if qq else nc.sync).dma_start(
                                out_d.ap()[P * tm:P * tm + P, c0:c0 + 256],
                                o_sb[:, H * ti + c0:H * ti + c0 + 256])
                        continue
                    for cj in range(2):
                        p_t = ps.tile([P, FD], F32, tag="big", bufs=8,
                                      name=f"pf{tm}{cj}")
                        for to in range(NT):
                            nc.tensor.matmul(
                                p_t[:],
                                qt[to][:, P * tm:P * tm + P],
                                rstack[to][:, FD * cj:FD * cj + FD],
                                start=(to == 0), stop=(to == NT - 1),
                            )
                        nc.vector.tensor_copy(
                            o_sb[:, H * ti + FD * cj:H * ti + FD * cj + FD],
                            p_t[:])
                if tp < NT // 2 - 1:
                    dst = out_d.ap()[256 * tp:256 * tp + 256, :] \
                        .rearrange("(t p) c -> p t c", p=P)
                    nc.sync.dma_start(
                        dst, o_sb[:].rearrange("p (t c) -> p t c", c=H))
                else:
                    # tm=6 alone; tm=7 went out as quarters above
                    nc.sync.dma_start(out_d.ap()[P * 6:P * 7, :],
                                      o_sb[:, 0:H])

    nc.compile()
    return nc


def _get_nc():
    if "nc" not in _CACHE:
        _CACHE["nc"] = _build()
    return _CACHE["nc"]


def _arrange(wt_f32):
    """[NT*P, H] row-tile layout -> [P, NT*H] one-descriptor layout."""
    bf16 = ml_dtypes.bfloat16
    a = np.ascontiguousarray(wt_f32).astype(bf16)
    return np.ascontiguousarray(
        a.reshape(NT, P, H).transpose(1, 0, 2).reshape(P, NT * H))


def _run(h, Wq, Wk, Wv, Wo, trace=False):
    nc = _get_nc()
    bf16 = ml_dtypes.bfloat16
    wkb = _arrange(np.asarray(Wk).T)
    wvb = _arrange(np.asarray(Wv).T)
    wqb = _arrange(np.asarray(Wq).T)
    wob = _arrange(np.asarray(Wo).T)
    in_maps = []
    for b in range(NC):
        hb = np.ascontiguousarray(np.asarray(h[b])).astype(bf16)
        htb = _arrange(np.asarray(h[b]).T)
        in_maps.append({
            "hn": hb, "htb": htb,
            "wkb": wkb, "wvb": wvb, "wqb": wqb, "wob": wob,
        })
    res = run_bass_kernel_spmd(nc, in_maps, core_ids=list(range(NC)),
                               trace=trace)
    out = np.stack(
        [res.results[b]["out"].astype(np.float32) for b in range(NC)], axis=0)
    return out, res


def kernel(h, key_pe, Wq, Wk, Wv, Wo):
    # key_pe only feeds the reference's dead softmax branch; unused.
    out, _ = _run(h, Wq, Wk, Wv, Wo)
    return out


# revision 59
# speedup vs baseline: 1.0558x; 1.0558x over previous
"""TRN2 Bass kernel for nn_MultiHeadSeqAttention (B=8, M=1024, H=1024, 16 heads).

Reference computes out = ((h Wq^T) (h Wk^T)^T) (h Wv^T) per head, then Wo^T.
No softmax, so the product reassociates per head h:
    out_h = q_h @ (k_h^T v_h)      with  k_h^T v_h = Wk_h (h^T h) Wv_h^T
Route through the Gram matrix G = h^T h, which is SYMMETRIC:
    G  = hN^T hN          upper-triangle windows only: 72 MM-equivalents
                          (28 lower [128,128] blocks via 7 batched DMA-XBAR
                          transposes on the otherwise idle scalar queue)
    AT = G @ wk           [j, o], 128 MMs                  (wk = Wk^T)
    S'_h = wv_h^T AT_h    per-head [64,64], head-pair packed, 64 small MMs
    qT = wq^T ht          128 MMs
    R_h = S_h @ wo_h      32 quadrant-paired MMs   (folds Wo into S)
    out = qT^T @ R        128 MMs
vs the direct route (k, v, q, Wo GEMMs = 512 MMs + smalls) this saves ~56
big-MM equivalents (~12us PE) per core.

Schedule notes (measured, from trace analysis):
 - every matmul lives in a same-PSUM-bank accumulation group so LDWEIGHTS
   hides behind the previous matmul (216ns/MM); per-MM bank cycling
   measured 474ns/MM (LDW exposed + HAM oscillation).
 - all input tensors stream on the sync queue as 7 descriptors in
   first-use order (hn halves, wk, wv, wq, ht, wo). A solo ring sustains
   ~235-280 GB/s; concurrent rings split unevenly (~240/110) and starve
   whoever is second; DMA elements >16KB run at ~105 GB/s (keep <=16KB).
   The completion-semaphore pool is ~10 deep and recycles in global
   emission order, so descriptor count stays minimal and the transposes
   recycle only early-completing hn semaphores.
 - AT visits jb=7 first: it needs no transposed G blocks and its S'
   matmuls (first touch of wv) land just after the wv load completes.
 - G pass A is paced by the two hn half-arrivals (4-MM bursts per bank);
   a const-operand warmup burst covers the HAM clock ramp until then.
 - R runs as one dense block after qT (interleaving cost ~1us); its
   PSUM evacuation alternates scalar/vector so casts never pace it.
 - output is bf16 (host casts back to f32), halving the store traffic;
   the last tile leaves as four [128,256] chunks in separate PSUM banks
   so the final cast+store chain is short.

Sharding: data-parallel over B across 8 cores; no collectives.
Precision: bf16 operands, fp32 PSUM accumulation; absmax rel err ~5.0e-3.
"""

import numpy as np
import ml_dtypes

import concourse.bass as bass
import concourse.mybir as mybir
import concourse.tile as tile
from concourse import bacc
from concourse.bass_utils import run_bass_kernel_spmd

F32 = mybir.dt.float32
BF16 = mybir.dt.bfloat16
COPY = mybir.ActivationFunctionType.Copy

P = 128          # partitions
H = 1024         # model dim
M = 1024         # sequence length
NT = H // P      # 8 tiles of 128
D = 64           # head dim
NC = 8           # cores
FD = 512         # matmul moving free dim (one PSUM bank of fp32)
WARMUP_MM = 10   # const-operand matmuls; cold pace fills until hn lands

_CACHE = {}


def _build():
    nc = bacc.Bacc("TRN2", target_bir_lowering=False, debug=False,
                   num_devices=NC, enable_asserts=False)

    hn_d = nc.dram_tensor("hn", [M, H], BF16, kind="ExternalInput")
    wkb_d = nc.dram_tensor("wkb", [P, NT * H], BF16, kind="ExternalInput")
    wvb_d = nc.dram_tensor("wvb", [P, NT * H], BF16, kind="ExternalInput")
    wqb_d = nc.dram_tensor("wqb", [P, NT * H], BF16, kind="ExternalInput")
    htb_d = nc.dram_tensor("htb", [P, NT * H], BF16, kind="ExternalInput")
    wob_d = nc.dram_tensor("wob", [P, NT * H], BF16, kind="ExternalInput")
    out_d = nc.dram_tensor("out", [M, H], BF16, kind="ExternalOutput")

    with tile.TileContext(nc) as tc:
        with tc.tile_pool(name="sb", bufs=1) as sb, \
             tc.tile_pool(name="ps", bufs=1, space="PSUM") as ps:

            # ---- warmup: dep-free matmuls; ends about when hn half 0 lands
            wu_lhs = nc.const_aps.tensor(1.0, [P, P], BF16)
            wu_rhs = nc.const_aps.tensor(1.0, [P, FD], BF16)
            wu_ps = ps.tile([P, FD], F32, tag="big", bufs=8, name="wu_ps")
            for _ in range(WARMUP_MM):
                nc.tensor.matmul(wu_ps[:], wu_lhs, wu_rhs,
                                 start=True, stop=True)
            # finer-grained tail so the handoff to the first G matmul
            # overshoots hn arrival by at most ~110ns
            for _ in range(6):
                nc.tensor.matmul(wu_ps[:, 0:P], wu_lhs, wu_rhs[:, 0:P],
                                 start=True, stop=True)

            # ---- loads: all sequential on the sync ring in first-use
            # order; the scalar ring is reserved for the G transposes ----
            hnBig = sb.tile([P, NT * H], BF16, tag="hn", name="hn")
            for r0, r1 in ((0, 512), (512, 1024)):
                src = hn_d.ap()[r0:r1, :].rearrange("(t p) c -> p t c", p=P)
                dst = hnBig[:, H * r0 // P:H * r1 // P] \
                    .rearrange("p (t c) -> p t c", c=H)
                nc.sync.dma_start(dst, src)

            def hn(s):
                return hnBig[:, H * s:H * s + H]

            def big_load(dram, tag, eng):
                t = sb.tile([P, NT * H], BF16, tag=tag, name=tag)
                eng.dma_start(t[:], dram.ap()[:, :])
                return t

            wkB = big_load(wkb_d, "wkB", nc.sync)
            wvB = big_load(wvb_d, "wvB", nc.sync)
            # wq/ht/wo are emitted AFTER the G transposes (below) so the
            # transposes' completion semaphores recycle only from the five
            # early loads; these three have >=15us of deadline slack

            # ---- phase G: upper-triangle windows of G = hN^T hN.
            # Row-chunk ib keeps columns [128*ib, 1024) as (up to) two PSUM
            # windows. Pass A = ib 0-3 (8 banks), 4-MM bursts per bank paced
            # by the hn halves; pass B = ib 4-7 (4 banks), dense. ----
            gt = sb.tile([P, NT * H], BF16, tag="gt", name="gt")
            # midpoint split keeps every window >=320 cols so LDWEIGHTS
            # (107ns) always hides under the matmul stream (>=133ns);
            # a 512/(512-128*ib) split leaves the narrow window LDW-bound
            WIN_A = []
            for ib in range(4):
                half = (H - P * ib) // 2 // 64 * 64
                WIN_A.append((ib, P * ib, half))
                WIN_A.append((ib, P * ib + half, H - P * ib - half))
            gpsA = {}
            for (ib, c0, w) in WIN_A:
                gpsA[(ib, c0)] = ps.tile([P, w], F32, tag="big", bufs=8,
                                         name=f"gA{ib}_{c0}")
            for s_range in (range(0, 4), range(4, 8)):
                for (ib, c0, w) in WIN_A:
                    for s in s_range:
                        nc.tensor.matmul(
                            gpsA[(ib, c0)][:],
                            hn(s)[:, P * ib:P * ib + P],
                            hn(s)[:, c0:c0 + w],
                            start=(s == 0), stop=(s == NT - 1),
                            skip_group_check=True,
                        )
            for (ib, c0, w) in WIN_A:
                nc.vector.tensor_copy(gt[:, H * ib + c0:H * ib + c0 + w],
                                      gpsA[(ib, c0)][:])
            for ib in range(4, 8):
                c0, w = P * ib, H - P * ib
                pt = ps.tile([P, w], F32, tag="big", bufs=8, name=f"gB{ib}")
                for s in range(NT):
                    nc.tensor.matmul(
                        pt[:],
                        hn(s)[:, P * ib:P * ib + P],
                        hn(s)[:, c0:c0 + w],
                        start=(s == 0), stop=(s == NT - 1),
                    )
                nc.vector.tensor_copy(gt[:, H * ib + c0:H * ib + c0 + w],
                                      pt[:])

            # lower-triangle blocks (b, a), b > a: batched XBAR transposes on
            # the scalar queue, one instruction per source row-chunk a,
            # ordered by when the AT phase consumes them (jb visit order).
            gt3 = gt[:].rearrange("p (b c) -> p b c", c=H)
            for a in (0, 1, 2, 3, 4, 5, 6):
                nc.scalar.dma_start(
                    gt3[:, a + 1:NT, P * a:P * a + P],
                    gt[:, H * a + P * (a + 1):H * a + H],
                    transpose=True)

            wqB = big_load(wqb_d, "wqB", nc.sync)
            htB = big_load(htb_d, "htB", nc.sync)
            woB = big_load(wob_d, "woB", nc.sync)

            # ---- phase AT + S': AT = G @ wk; S' pairs accumulate after each
            # at-tile half is cast. jb=7 needs no transposed blocks; jb 3..0
            # need pass-A-sourced transposes, jb 6..4 pass-B-sourced. ----
            s_psA = ps.tile([P, FD], F32, tag="big", bufs=8, name="s_psA")
            s_psB = ps.tile([P, FD], F32, tag="big", bufs=8, name="s_psB")
            nc.vector.memset(s_psA[:], 0.0)
            nc.vector.memset(s_psB[:], 0.0)

            # jb=7 first: it needs no transposed blocks, and its S' matmuls
            # (the first to touch wv) land just after the wv load completes
            AT_ORDER = [7, 3, 2, 1, 0, 6, 5, 4]
            for idx, jb in enumerate(AT_ORDER):
                a_t = sb.tile([P, H], BF16, tag="at", bufs=3, name=f"at{jb}")
                for oc in range(2):
                    p_t = ps.tile([P, FD], F32, tag="big", bufs=8,
                                  name=f"pa{jb}{oc}")
                    for ib in range(NT):
                        nc.tensor.matmul(
                            p_t[:],
                            gt[:, H * ib + P * jb:H * ib + P * jb + P],
                            wkB[:, H * ib + FD * oc:H * ib + FD * oc + FD],
                            start=(ib == 0), stop=(ib == NT - 1),
                        )
                    nc.vector.tensor_copy(a_t[:, FD * oc:FD * oc + FD], p_t[:])
                    for g in range(4 * oc, 4 * oc + 4):
                        bank = s_psA if g < 4 else s_psB
                        cc = P * (g % 4)
                        nc.tensor.matmul(
                            bank[:, cc:cc + P],
                            wvB[:, H * jb + P * g:H * jb + P * g + P],
                            a_t[:, P * g:P * g + P],
                            start=False, stop=(idx == NT - 1),
                            skip_group_check=True,
                        )
            s_sbA = sb.tile([P, FD], BF16, tag="ssb", bufs=2, name="s_sbA")
            s_sbB = sb.tile([P, FD], BF16, tag="ssb", bufs=2, name="s_sbB")
            nc.scalar.activation(s_sbA[:], s_psA[:], COPY)
            nc.scalar.activation(s_sbB[:], s_psB[:], COPY)

            # ---- phase qT + R: qT = wq^T ht; R_g folds Wo into S per head
            # pair (quadrant-packed 64-part matmuls), spread between q tiles ----
            qt = [None] * NT
            rstack = [None] * NT

            def emit_q(to):
                q_t = sb.tile([P, M], BF16, tag=f"qt{to}", name=f"qt{to}")
                for cm in range(2):
                    p_t = ps.tile([P, FD], F32, tag="big", bufs=8,
                                  name=f"pq{to}{cm}")
                    for ci in range(NT):
                        nc.tensor.matmul(
                            p_t[:],
                            wqB[:, H * ci + P * to:H * ci + P * to + P],
                            htB[:, H * ci + FD * cm:H * ci + FD * cm + FD],
                            start=(ci == 0), stop=(ci == NT - 1),
                        )
                    nc.vector.tensor_copy(q_t[:, FD * cm:FD * cm + FD], p_t[:])
                qt[to] = q_t

            def emit_r(g):
                r_t = sb.tile([P, H], BF16, tag=f"rs{g}", name=f"rs{g}")
                sbank = s_sbA if g < 4 else s_sbB
                cc = P * (g % 4)
                for jc in range(2):
                    p_t = ps.tile([P, FD], F32, tag="big", bufs=8,
                                  name=f"pr{g}{jc}")
                    for hh in range(2):
                        pb = D * hh
                        nc.tensor.matmul(
                            p_t[pb:pb + D, :],
                            sbank[pb:pb + D, cc + pb:cc + pb + D],
                            woB[pb:pb + D, H * g + FD * jc:H * g + FD * jc + FD],
                            start=True, stop=True,
                        )
                    if jc:
                        nc.vector.tensor_copy(
                            r_t[:, FD * jc:FD * jc + FD], p_t[:])
                    else:
                        nc.scalar.activation(r_t[:, FD * jc:FD * jc + FD],
                                             p_t[:], COPY)
                rstack[g] = r_t

            # R as one dense block after qT: interleaving it between q tiles
            # measured ~1us of group-switch overhead
            for to in range(NT):
                emit_q(to)
            for g in range(NT):
                emit_r(g)

            # ---- phase out: out = qT^T @ R. Stores are paired (two m-tiles
            # per descriptor) to keep the DMA instruction count low; the
            # last tile goes out as quarter chunks so the end of kernel
            # gates on a 64KB transfer. ----
            for tp in range(NT // 2):
                o_sb = sb.tile([P, 2 * H], BF16, tag="ot", bufs=2,
                               name=f"o{tp}")
                for ti in range(2):
                    tm = 2 * tp + ti
                    if tm == NT - 1:
                        # last tile: cj=0 as a normal [128,512] group+store;
                        # cj=1 as two [128,256] groups in separate PSUM
                        # banks so the final cast+store chain is short
                        p_t = ps.tile([P, FD], F32, tag="big", bufs=8,
                                      name="pf7c0")
                        for to in range(NT):
                            nc.tensor.matmul(
                                p_t[:],
                                qt[to][:, P * tm:P * tm + P],
                                rstack[to][:, 0:FD],
                                start=(to == 0), stop=(to == NT - 1),
                            )
                        nc.vector.tensor_copy(
                            o_sb[:, H * ti:H * ti + FD], p_t[:])
                        nc.sync.dma_start(
                            out_d.ap()[P * tm:P * tm + P, 0:FD],
                            o_sb[:, H * ti:H * ti + FD])
                        for qq in range(2):
                            c0 = FD + 256 * qq
                            p_t = ps.tile([P, 256], F32, tag="big", bufs=8,
                                          name=f"pf7q{qq}")
                            for to in range(NT):
                                nc.tensor.matmul(
                                    p_t[:],
                                    qt[to][:, P * tm:P * tm + P],
                                    rstack[to][:, c0:c0 + 256],
                                    start=(to == 0), stop=(to == NT - 1),
                                )
                            nc.vector.tensor_copy(
                                o_sb[:, H * ti + c0:H * ti + c0 + 256],
                                p_t[:])
                            (nc.scalar if qq else nc.sync).dma_start(
                                out_d.ap()[P * tm:P * tm + P, c0:c0 + 256],
                                o_sb[:, H * ti + c0:H * ti + c0 + 256])
                        continue
                    for cj in range(2):
                        p_t = ps.tile([P, FD], F32, tag="big", bufs=8,
                                      name=f"pf{tm}{cj}")
                        for to in range(NT):
                            nc.tensor.matmul(
                                p_t[:],
                                qt[to][:, P * tm:P * tm + P],
                                rstack[to][:, FD * cj:FD * cj + FD],
                                start=(to == 0), stop=(to == NT - 1),
                            )
                        nc.vector.tensor_copy(
                            o_sb[:, H * ti + FD * cj:H * ti + FD * cj + FD],
                            p_t[:])
                if tp < NT // 2 - 1:
                    dst = out_d.ap()[256 * tp:256 * tp + 256, :] \
                        .rearrange("(t p) c -> p t c", p=P)
                    nc.sync.dma_start(
                        dst, o_sb[:].rearrange("p (t c) -> p t c", c=H))
                else:
                    # tm=6 alone; tm=7 went out as quarters above
                    nc.sync.dma_start(out_d.ap()[P * 6:P * 7, :],
                                      o_sb[:, 0:H])

    nc.compile()
    return nc


def _get_nc():
    if "nc" not in _CACHE:
        _CACHE["nc"] = _build()
    return _CACHE["nc"]


def _arrange(wt_f32):
    """[NT*P, H] row-tile layout -> [P, NT*H] one-descriptor layout."""
    bf16 = ml_dtypes.bfloat16
    a = np.ascontiguousarray(wt_f32).astype(bf16)
    return np.ascontiguousarray(
        a.reshape(NT, P, H).transpose(1, 0, 2).reshape(P, NT * H))


def _run(h, Wq, Wk, Wv, Wo, trace=False):
    nc = _get_nc()
    bf16 = ml_dtypes.bfloat16
    wkb = _arrange(np.asarray(Wk).T)
    wvb = _arrange(np.asarray(Wv).T)
    wqb = _arrange(np.asarray(Wq).T)
    wob = _arrange(np.asarray(Wo).T)
    in_maps = []
    for b in range(NC):
        hb = np.ascontiguousarray(np.asarray(h[b])).astype(bf16)
        htb = _arrange(np.asarray(h[b]).T)
        in_maps.append({
            "hn": hb, "htb": htb,
            "wkb": wkb, "wvb": wvb, "wqb": wqb, "wob": wob,
        })
    res = run_bass_kernel_spmd(nc, in_maps, core_ids=list(range(NC)),
                               trace=trace)
    out = np.stack(
        [res.results[b]["out"].astype(np.float32) for b in range(NC)], axis=0)
    return out, res


def kernel(h, key_pe, Wq, Wk, Wv, Wo):
    # key_pe only feeds the reference's dead softmax branch; unused.
    out, _ = _run(h, Wq, Wk, Wv, Wo)
    return out
